# revision 1
# baseline (speedup 1.0000x reference)
"""Trainium2 Bass kernel for nn_Conv2D_80796924772741.

Depthwise (grouped, F=64) 3x3 valid conv over [F, 514, 514, 4] int8 with
per-channel int8 weights + int32 bias, followed by exact fixed-point requant
  res = (acc * 19920 + 2^21) >> 22 ;  out = clip(res - 5, -128, 127) int8
(reduced_mantissa 19920 = 1245 * 16 -> res = (acc*1245 + 2^17) >> 18).

Sharding: F=64 split across 8 NeuronCores (8 channels each), embarrassingly
parallel.

Per-core compute:
 - PE: per channel, conv via Toeplitz-band stationary matmuls over H-windows
   (contraction = 128 input rows; all 3 H-taps in the band diagonals; 3
   matmuls for the 3 W-taps, W-shift = +4n free-dim offset since (w,d) is
   flattened). Bias b and a -63.5 rounding offset ride two all-ones rhs
   partitions with per-output-column weights.  PSUM accA = conv + b - 63.5.
 - ACT: h'' = fma(accA * 2^-7 + 1.5*2^23)  == 1.5*2^23 + floor(acc/128)
   (exact: RNE at ulp=1, offset -63.5 centers the fraction, never ties).
 - DVE: hi = h'' - 1.5*2^23  (exact, fits fp16)
 - PE: accA += (-128*I) @ hi  -> l' = lo - 63.5  (lo = acc mod 128)
 - ACT: g = fma(l' * (1245/128) - 8598.861328125)   [= gamma - c2 - 9216]
 - DVE: f5 = RNE(g)          (magic-add pair)      [= floor(gamma) - 9216]
        S  = hi*1245 + f5    (scalar_tensor_tensor; exact, < 2^21)
        v  = S*2^-11 - 0.499755859375
        r  = RNE(v)                                 [= res - 5]
        out = clip(r, -128, 127) -> int8
Every intermediate is exactly representable in fp32; the chain was verified
bit-exact against the int64 reference over the full accumulator range.
"""

import numpy as np
import ml_dtypes

F_PER_CORE = 8
H_IN = 514
W_IN = 514
D = 4
H_OUT = 512
WD_OUT = 2048  # 512 * 4
FREE_IN = W_IN * D  # 2056
N_CHUNK = 512
N_CORES = 8

# H windows: output rows per window (partition-limited: K = M + 4 <= 128)
WINDOWS = [(0, 124), (124, 124), (248, 124), (372, 124), (496, 16)]

MAGIC = 12582912.0  # 1.5 * 2^23 : RNE-to-integer magic for |x| < 2^22


def _build_lhsT(w_core: np.ndarray, b_core: np.ndarray) -> np.ndarray:
    """[128, 8*3*124] bf16 stationary: per (channel, w-tap) a Toeplitz band.

    Layout column block (f*3 + n)*124 : +124  holds T_n for channel f.
    T_n[2 + i + m, i] = w[f, m, n]  (rows 2.. are conv data partitions)
    T_0[0, i] = 8*floor(b/8) ; T_0[1, i] = (b mod 8) - 63.5  (bias rows,
    multiplied by all-ones rhs partitions 0/1).
    """
    out = np.zeros((128, F_PER_CORE * 3 * 124), dtype=np.float32)
    for f in range(F_PER_CORE):
        b_f = int(b_core[f])
        bh = b_f >> 3  # floor division
        bl = b_f - 8 * bh
        for n in range(3):
            base = (f * 3 + n) * 124
            if n == 0:
                out[0, base : base + 124] = float(8 * bh)
                out[1, base : base + 124] = float(bl) - 63.5
            for m in range(3):
                wv = float(int(w_core[f, m, n, 0]))
                # T[2 + i + m, base + i] = wv  for i in 0..123
                idx = np.arange(124)
                out[2 + idx + m, base + idx] = wv
    return out.astype(ml_dtypes.bfloat16)


_PROGRAM_CACHE = {}


def _build_program():
    import concourse.bass as bass
    import concourse.tile as tile
    from concourse import bacc, mybir

    nc = bacc.Bacc(
        "TRN2", target_bir_lowering=False, debug=False, num_devices=N_CORES
    )
    dt = mybir.dt
    Alu = mybir.AluOpType
    Act = mybir.ActivationFunctionType

    x_d = nc.dram_tensor(
        "x", [F_PER_CORE, H_IN, FREE_IN], dt.int8, kind="ExternalInput"
    ).ap()
    lhsT_d = nc.dram_tensor(
        "lhsT", [128, F_PER_CORE * 3 * 124], dt.bfloat16, kind="ExternalInput"
    ).ap()
    id_d = nc.dram_tensor("id4", [124, 124], dt.float16, kind="ExternalInput").ap()
    ones_d = nc.dram_tensor("ones2", [2, FREE_IN], dt.bfloat16, kind="ExternalInput").ap()
    y_d = nc.dram_tensor(
        "y", [F_PER_CORE, H_OUT, WD_OUT], dt.int8, kind="ExternalOutput"
    ).ap()

    with tile.TileContext(nc) as tc:
        with (
            tc.tile_pool(name="const", bufs=1) as const_pool,
            tc.tile_pool(name="xin", bufs=3) as x_pool,
            tc.tile_pool(name="psum", bufs=6, space="PSUM") as psum_pool,
            tc.tile_pool(name="hbig", bufs=3) as h_pool,
            tc.tile_pool(name="hi16", bufs=3) as hi_pool,
            tc.tile_pool(name="gtile", bufs=3) as g_pool,
            tc.tile_pool(name="ftile", bufs=3) as f_pool,
            tc.tile_pool(name="stile", bufs=3) as s_pool,
            tc.tile_pool(name="vtile", bufs=3) as v_pool,
            tc.tile_pool(name="rtile", bufs=3) as r_pool,
            tc.tile_pool(name="otile", bufs=3) as o_pool,
        ):
            lhsT_t = const_pool.tile([128, F_PER_CORE * 3 * 124], dt.bfloat16)
            nc.sync.dma_start(lhsT_t[:], lhsT_d[:])
            id_t = const_pool.tile([124, 124], dt.float16)
            nc.sync.dma_start(id_t[:], id_d[:])

            for f in range(F_PER_CORE):
                for (r0, m_r) in WINDOWS:
                    k_r = m_r + 4  # 2 ones rows + m_r + 2 data rows
                    xt = x_pool.tile([128, FREE_IN], dt.bfloat16)
                    # ones rows (bias partitions)
                    nc.sync.dma_start(xt[0:2, :], ones_d[:])
                    # data rows with int8 -> bf16 cast (SWDGE)
                    nc.gpsimd.dma_start(
                        xt[2 : 2 + m_r + 2, :], x_d[f, r0 : r0 + m_r + 2, :]
                    )
                    for c in range(4):
                        ps = psum_pool.tile([124, N_CHUNK], dt.float32)
                        for n in range(3):
                            base = (f * 3 + n) * 124
                            nc.tensor.matmul(
                                ps[0:m_r, :],
                                lhsT_t[0:k_r, base : base + m_r],
                                xt[0:k_r, c * N_CHUNK + 4 * n : c * N_CHUNK + 4 * n + N_CHUNK],
                                start=(n == 0),
                                stop=False,
                                skip_group_check=True,
                            )
                        # h'' = 1.5*2^23 + floor(acc/128)
                        ht = h_pool.tile([124, N_CHUNK], dt.float32)
                        nc.scalar.activation(
                            ht[0:m_r, :], ps[0:m_r, :], Act.Copy,
                            bias=MAGIC, scale=0.0078125,
                        )
                        hit = hi_pool.tile([124, N_CHUNK], dt.float16)
                        nc.vector.tensor_scalar(
                            hit[0:m_r, :], ht[0:m_r, :], -MAGIC, None, Alu.add
                        )
                        # accA += -128 * hi  -> l' = (acc mod 128) - 63.5
                        nc.tensor.matmul(
                            ps[0:m_r, :],
                            id_t[0:m_r, 0:m_r],
                            hit[0:m_r, :],
                            start=False,
                            stop=True,
                            skip_group_check=True,
                        )
                        # g = gamma - c2 - 9216
                        gt = g_pool.tile([124, N_CHUNK], dt.float32)
                        nc.scalar.activation(
                            gt[0:m_r, :], ps[0:m_r, :], Act.Copy,
                            bias=-8598.861328125, scale=9.7265625,
                        )
                        # f5 = RNE(g) = floor(gamma) - 9216
                        ft = f_pool.tile([124, N_CHUNK], dt.float32)
                        nc.vector.tensor_scalar(
                            ft[0:m_r, :], gt[0:m_r, :], MAGIC, -MAGIC, Alu.add, Alu.add
                        )
                        # S = hi*1245 + f5
                        st = s_pool.tile([124, N_CHUNK], dt.float32)
                        nc.vector.scalar_tensor_tensor(
                            st[0:m_r, :], hit[0:m_r, :], 1245.0, ft[0:m_r, :],
                            Alu.mult, Alu.add,
                        )
                        # v = S*2^-11 - (0.5 - 2^-12)
                        vt = v_pool.tile([124, N_CHUNK], dt.float32)
                        nc.vector.tensor_scalar(
                            vt[0:m_r, :], st[0:m_r, :], 0.00048828125,
                            0.499755859375, Alu.mult, Alu.subtract,
                        )
                        # r = RNE(v) = res - 5
                        rt = r_pool.tile([124, N_CHUNK], dt.float32)
                        nc.vector.tensor_scalar(
                            rt[0:m_r, :], vt[0:m_r, :], MAGIC, -MAGIC, Alu.add, Alu.add
                        )
                        # clip to [-128, 127] -> int8
                        ot = o_pool.tile([124, N_CHUNK], dt.int8)
                        nc.vector.tensor_scalar(
                            ot[0:m_r, :], rt[0:m_r, :], -128.0, 127.0, Alu.max, Alu.min
                        )
                        nc.sync.dma_start(
                            y_d[f, r0 : r0 + m_r, c * N_CHUNK : (c + 1) * N_CHUNK],
                            ot[0:m_r, :],
                        )

    nc.compile()
    return nc


def kernel(x: np.ndarray, w: np.ndarray, b: np.ndarray) -> np.ndarray:
    """x: int8 [64, 514, 514, 4]; w: int8 [64, 3, 3, 1]; b: int32 [64].

    Returns int8 [64, 512, 512, 4].
    """
    from concourse.bass_utils import run_bass_kernel_spmd

    if "nc" not in _PROGRAM_CACHE:
        _PROGRAM_CACHE["nc"] = _build_program()
    nc = _PROGRAM_CACHE["nc"]

    F = x.shape[0]
    assert F == N_CORES * F_PER_CORE

    id4 = (-128.0 * np.eye(124, dtype=np.float32)).astype(np.float16)
    ones2 = np.ones((2, FREE_IN), dtype=np.float32).astype(ml_dtypes.bfloat16)

    in_maps = []
    for core in range(N_CORES):
        lo = core * F_PER_CORE
        hi = lo + F_PER_CORE
        x_shard = np.ascontiguousarray(x[lo:hi]).reshape(F_PER_CORE, H_IN, FREE_IN)
        lhsT = _build_lhsT(w[lo:hi], b[lo:hi])
        in_maps.append({"x": x_shard, "lhsT": lhsT, "id4": id4, "ones2": ones2})

    res = run_bass_kernel_spmd(nc, in_maps, core_ids=list(range(N_CORES)))

    out = np.empty((F, H_OUT, 512, D), dtype=np.int8)
    for core in range(N_CORES):
        lo = core * F_PER_CORE
        y = res.results[core]["y"]  # [8, 512, 2048] int8
        out[lo : lo + F_PER_CORE] = y.reshape(F_PER_CORE, H_OUT, 512, D)
    return out



# revision 6
# speedup vs baseline: 1.7755x; 1.7755x over previous
"""Trainium2 Bass kernel for nn_Conv2D_80796924772741.

Depthwise (grouped, F=64) 3x3 valid conv over [F, 514, 514, 4] int8 with
per-channel int8 weights + int32 bias, followed by exact fixed-point requant
  acc = conv + b ;  res = (acc*1245 + 2^17) >> 18
  out = clip(res - 5, -128, 127) int8

Sharding: F=64 split across 8 NeuronCores (8 channels each), embarrassingly
parallel.

Per-core compute:
 - PE: per channel, conv via Toeplitz-band stationary matmuls over H-windows
   (contraction = 128 rows: 2 ones rows carrying the int32 bias b split as
   8*floor(b/8) + (b mod 8), then 126 data rows; 3 matmuls for the 3 W-taps,
   W-shift = +4n free-dim offset since (w,d) is flattened). PSUM [124, 2048]
   spans 4 banks; 4 chunks x 3 taps accumulate per window, so PSUM = acc.
 - Requant in 3 elementwise ops, exploiting the HW's RNE+saturate conversion
   on every fp32->int write (verified on-device). With 1245 = 5*256 - 35 and
   K0 = 2^17 - 5*2^18 (folds the rounding bias and the -5 zero point):
     E   = floor((K0 - 35*acc)/256)       [ACT: acc*(-35/256) - 4608.498046875
                                           -> int16; RNE w/ -255/512 centering
                                           == exact floor]
     S   = 5*acc + E                      [DVE scalar_tensor_tensor from PSUM
                                           -> int32, exact in fp32]
     out = clip(floor(S/1024), -128, 127) [S*2^-10 - 0.49951171875 -> int8;
                                           RNE+saturate == floor + clip]
   Every intermediate is exactly representable in fp32; the chain was
   verified bit-exact against the int64 reference over the full acc range.
   The final op alternates DVE/ACT per window to balance engine load.
 - Output rows 496..511: two block-diagonal matmuls (4 channels each,
   74 contraction rows -> 64 outputs) reuse the same requant.
"""

import numpy as np
import ml_dtypes

F_PER_CORE = 8
H_IN = 514
W_IN = 514
D = 4
H_OUT = 512
WD_OUT = 2048  # 512 * 4
FREE_IN = W_IN * D  # 2056
N_CHUNK = 512
N_CORES = 8

M_R = 124                      # output rows per full window
N_WIN = 4                      # full windows: rows 0..495
M_LEFT = 16                    # leftover rows per channel (496..511)
K_LEFT = M_LEFT + 2            # 18 data rows per channel in leftover MM
CH_HALF = 4                    # channels per leftover MM

SCALE_E = -0.13671875          # -35/256
BIAS_A = -4608.498046875       # (2*K0 - 255)/512, K0 = 2^17 - 5*2^18
SCALE_S = 0.0009765625         # 2^-10
BIAS_D = 0.49951171875         # 0.5 - 2^-11 (subtracted)


def _bias_rows(b: int):
    bh = int(b) >> 3
    return float(8 * bh), float(int(b) - 8 * bh)


def _build_lhsT(w_core: np.ndarray, b_core: np.ndarray) -> np.ndarray:
    """[128, 8*3*124] bf16 stationary: per (channel, w-tap) a Toeplitz band.

    Column block (f*3 + n)*124 : +124 holds T_n for channel f:
    T_n[2 + i + m, i] = w[f, m, n]  (rows 2..127 are conv data partitions).
    T_0[0, i] = 8*floor(b/8) ; T_0[1, i] = b mod 8  (bias rows, multiplied by
    all-ones rhs partitions 0/1).
    """
    out = np.zeros((128, F_PER_CORE * 3 * M_R), dtype=np.float32)
    idx = np.arange(M_R)
    for f in range(F_PER_CORE):
        bh8, bl = _bias_rows(int(b_core[f]))
        for n in range(3):
            base = (f * 3 + n) * M_R
            if n == 0:
                out[0, base : base + M_R] = bh8
                out[1, base : base + M_R] = bl
            for m in range(3):
                out[2 + idx + m, base + idx] = float(int(w_core[f, m, n, 0]))
    return out.astype(ml_dtypes.bfloat16)


def _build_lhsT_left(w_core: np.ndarray, b_core: np.ndarray, half: int) -> np.ndarray:
    """[74, 3*64] bf16 block-diagonal stationary for leftover rows 496..511.

    Channels 4*half..4*half+3. Tap-n block at cols n*64:
    T[2 + 18*j + i + m, n*64 + 16*j + i] = w[f, m, n]  (j = f - 4*half, i<16)
    plus bias rows 0/1 on the n==0 block.
    """
    out = np.zeros((2 + CH_HALF * K_LEFT, 3 * CH_HALF * M_LEFT), dtype=np.float32)
    idx = np.arange(M_LEFT)
    for j in range(CH_HALF):
        f = CH_HALF * half + j
        bh8, bl = _bias_rows(int(b_core[f]))
        for n in range(3):
            base = n * CH_HALF * M_LEFT + j * M_LEFT
            if n == 0:
                out[0, base : base + M_LEFT] = bh8
                out[1, base : base + M_LEFT] = bl
            for m in range(3):
                out[2 + j * K_LEFT + idx + m, base + idx] = float(
                    int(w_core[f, m, n, 0])
                )
    return out.astype(ml_dtypes.bfloat16)


_PROGRAM_CACHE = {}


def _build_program():
    import concourse.bass as bass
    import concourse.tile as tile
    from concourse import bacc, mybir

    nc = bacc.Bacc(
        "TRN2", target_bir_lowering=False, debug=False, num_devices=N_CORES
    )
    dt = mybir.dt
    Alu = mybir.AluOpType
    Act = mybir.ActivationFunctionType

    x_d = nc.dram_tensor(
        "x", [F_PER_CORE, H_IN, FREE_IN], dt.int8, kind="ExternalInput"
    ).ap()
    lhsT_d = nc.dram_tensor(
        "lhsT", [128, F_PER_CORE * 3 * M_R], dt.bfloat16, kind="ExternalInput"
    ).ap()
    lhsTl_d = nc.dram_tensor(
        "lhsTl", [2, 2 + CH_HALF * K_LEFT, 3 * CH_HALF * M_LEFT], dt.bfloat16,
        kind="ExternalInput",
    ).ap()
    ones_d = nc.dram_tensor("ones2", [2, FREE_IN], dt.bfloat16, kind="ExternalInput").ap()
    y_d = nc.dram_tensor(
        "y", [F_PER_CORE, H_OUT, WD_OUT], dt.int8, kind="ExternalOutput"
    ).ap()

    KL = 2 + CH_HALF * K_LEFT  # 74

    with tile.TileContext(nc) as tc:
        with (
            tc.tile_pool(name="const", bufs=1) as const_pool,
            tc.tile_pool(name="xin", bufs=3) as x_pool,
            tc.tile_pool(name="psum", bufs=2, space="PSUM") as psum_pool,
            tc.tile_pool(name="etile", bufs=2) as e_pool,
            tc.tile_pool(name="stile", bufs=2) as s_pool,
            tc.tile_pool(name="otile", bufs=2) as o_pool,
        ):
            lhsT_t = const_pool.tile([128, F_PER_CORE * 3 * M_R], dt.bfloat16)
            nc.sync.dma_start(lhsT_t[:], lhsT_d[:])
            lhsTl_t = const_pool.tile([KL, 2 * 3 * CH_HALF * M_LEFT], dt.bfloat16)
            nc.sync.dma_start(
                lhsTl_t[:, 0 : 3 * CH_HALF * M_LEFT], lhsTl_d[0, :, :]
            )
            nc.sync.dma_start(
                lhsTl_t[:, 3 * CH_HALF * M_LEFT :], lhsTl_d[1, :, :]
            )

            def requant(ps_ap, rows, alt):
                """PSUM acc -> E16 (ACT) -> S32 (DVE stt) -> out8."""
                et = e_pool.tile([M_R, WD_OUT], dt.int16)
                nc.scalar.activation(
                    et[0:rows, :], ps_ap, Act.Copy, bias=BIAS_A, scale=SCALE_E
                )
                st = s_pool.tile([M_R, WD_OUT], dt.int32)
                nc.vector.scalar_tensor_tensor(
                    st[0:rows, :], ps_ap, 5.0, et[0:rows, :], Alu.mult, Alu.add
                )
                ot = o_pool.tile([M_R, WD_OUT], dt.int8)
                if alt:
                    nc.scalar.activation(
                        ot[0:rows, :], st[0:rows, :], Act.Copy,
                        bias=-BIAS_D, scale=SCALE_S,
                    )
                else:
                    nc.vector.tensor_scalar(
                        ot[0:rows, :], st[0:rows, :], SCALE_S, BIAS_D,
                        Alu.mult, Alu.subtract,
                    )
                return ot

            wi = 0
            for f in range(F_PER_CORE):
                for w in range(N_WIN):
                    r0 = w * M_R
                    xt = x_pool.tile([128, FREE_IN], dt.bfloat16, tag="xt")
                    nc.sync.dma_start(xt[0:2, :], ones_d[:])
                    nc.gpsimd.dma_start(xt[2:128, :], x_d[f, r0 : r0 + M_R + 2, :])
                    ps = psum_pool.tile([M_R, WD_OUT], dt.float32, tag="ps")
                    for c in range(4):
                        for n in range(3):
                            base = (f * 3 + n) * M_R
                            nc.tensor.matmul(
                                ps[:, c * N_CHUNK : (c + 1) * N_CHUNK],
                                lhsT_t[:, base : base + M_R],
                                xt[:, c * N_CHUNK + 4 * n : c * N_CHUNK + 4 * n + N_CHUNK],
                                start=(n == 0),
                                stop=(n == 2),
                                skip_group_check=True,
                            )
                    ot = requant(ps[:], M_R, alt=(wi % 2 == 1))
                    nc.sync.dma_start(y_d[f, r0 : r0 + M_R, :], ot[0:M_R, :])
                    wi += 1

            # leftover rows 496..511: two block-diag MMs of 4 channels each
            for half in range(2):
                xl = x_pool.tile([128, FREE_IN], dt.bfloat16, tag="xt")
                nc.sync.dma_start(xl[0:2, :], ones_d[:])
                nc.gpsimd.dma_start(
                    xl[2:KL, :],
                    x_d[CH_HALF * half : CH_HALF * half + CH_HALF,
                        N_WIN * M_R : N_WIN * M_R + K_LEFT, :],
                )
                rows = CH_HALF * M_LEFT  # 64
                psl = psum_pool.tile([M_R, WD_OUT], dt.float32, tag="ps")
                for c in range(4):
                    for n in range(3):
                        base = half * 3 * rows + n * rows
                        nc.tensor.matmul(
                            psl[0:rows, c * N_CHUNK : (c + 1) * N_CHUNK],
                            lhsTl_t[:, base : base + rows],
                            xl[0:KL, c * N_CHUNK + 4 * n : c * N_CHUNK + 4 * n + N_CHUNK],
                            start=(n == 0),
                            stop=(n == 2),
                            skip_group_check=True,
                        )
                otl = requant(psl[0:rows, :], rows, alt=(half == 1))
                nc.sync.dma_start(
                    y_d[CH_HALF * half : CH_HALF * half + CH_HALF,
                        N_WIN * M_R : H_OUT, :],
                    otl[0:rows, :],
                )

    nc.compile()
    return nc


def make_in_maps(x: np.ndarray, w: np.ndarray, b: np.ndarray) -> list:
    ones2 = np.ones((2, FREE_IN), dtype=np.float32).astype(ml_dtypes.bfloat16)
    in_maps = []
    for core in range(N_CORES):
        lo = core * F_PER_CORE
        hi = lo + F_PER_CORE
        x_shard = np.ascontiguousarray(x[lo:hi]).reshape(F_PER_CORE, H_IN, FREE_IN)
        lhsT = _build_lhsT(w[lo:hi], b[lo:hi])
        lhsTl = np.stack(
            [_build_lhsT_left(w[lo:hi], b[lo:hi], h) for h in range(2)]
        )
        in_maps.append(
            {"x": x_shard, "lhsT": lhsT, "lhsTl": lhsTl, "ones2": ones2}
        )
    return in_maps


def kernel(x: np.ndarray, w: np.ndarray, b: np.ndarray) -> np.ndarray:
    """x: int8 [64, 514, 514, 4]; w: int8 [64, 3, 3, 1]; b: int32 [64].

    Returns int8 [64, 512, 512, 4].
    """
    from concourse.bass_utils import run_bass_kernel_spmd

    if "nc" not in _PROGRAM_CACHE:
        _PROGRAM_CACHE["nc"] = _build_program()
    nc = _PROGRAM_CACHE["nc"]

    F = x.shape[0]
    assert F == N_CORES * F_PER_CORE

    res = run_bass_kernel_spmd(nc, make_in_maps(x, w, b), core_ids=list(range(N_CORES)))

    out = np.empty((F, H_OUT, 512, D), dtype=np.int8)
    for core in range(N_CORES):
        lo = core * F_PER_CORE
        y = res.results[core]["y"]  # [8, 512, 2048] int8
        out[lo : lo + F_PER_CORE] = y.reshape(F_PER_CORE, H_OUT, 512, D)
    return out
